# revision 1
# baseline (speedup 1.0000x reference)
"""Trainium2 Bass kernel for nn_Attention — v2 (fp8 DoubleRow + dual-engine exp).

Sharding: 8 cores = 4 batches x 2 query-halves (800 queries each); each core
sees all 1600 keys. No collectives.

Cost-model-driven design:
  - Attention matmuls (QK, AV) and the Q/K/V-for-attention 1x1 convs run in
    fp8 (e4m3 weights/operands, e5m2 exp values) with perf_mode=DoubleRow:
    keys are processed in 256-wide super-blocks laid out [128 partitions,
    2 pair] so both QK and AV contract 256 elements per pass.
  - exp(logits) is split across ACT (true Exp -> e5m2 out) and DVE (Schraudolph
    bit-trick: bits = rint(x*scale*4/ln2 + 60) as uint8 == float8_e5m2).
  - dw conv branch, V-for-dw, and the final projection stay bf16 (fp8 there
    blows the error budget; attention-side fp8 is diluted ~8x by the dw
    branch).
  - Softmax denominator rides as a 65th V column (ones); normalization:
    reciprocal on DVE, PE broadcast into psum rows 64:128, DVE multiply.
  - BN folded on host; K-bias dropped (softmax-invariant); V-bias applied
    via rank-10 maug correction (baseline trick).
"""
import os
import sys

sys.path.insert(0, "/opt/trn_rl_repo")

import numpy as np
import ml_dtypes

BF = ml_dtypes.bfloat16
E4 = ml_dtypes.float8_e4m3
E5 = ml_dtypes.float8_e5m2

CH = 256
NH = 4
DH = 64
DK = 32
EPS = 1e-3
B = 4
H = W = 40
HW = 1600
NQ = 800
NCORES = 8
SCALE = DK ** -0.5
LN2 = float(np.log(2.0))

# key super-blocks: (index, base, nk); 6 full 256-blocks + one 64 tail
SUPERS = [(s, 256 * s, 256) for s in range(6)] + [(6, 1536, 64)]
CHUNKS = [tuple(map(int, c.split(":"))) for c in os.environ.get(
    "KERNEL_CHUNKS", "0:256,256:512,512:768,768:800").split(",")]
Q_CHUNKS = [(0, 400), (400, 800)]
K_CHUNKS = [(0, 400), (400, 800), (800, 1200), (1200, 1600)]

_COMPILED = None
LAST_EXEC_NS = None
LAST_RESULTS = None


def build_program():
    import concourse.bass as bass
    import concourse.bacc as bacc
    import concourse.tile as tile
    from concourse import mybir

    f32 = mybir.dt.float32
    bf16 = mybir.dt.bfloat16
    e4 = mybir.dt.float8e4
    e5 = mybir.dt.float8e5
    u8 = mybir.dt.uint8
    DR = mybir.MatmulPerfMode.DoubleRow
    Exp = mybir.ActivationFunctionType.Exp
    Ident = mybir.ActivationFunctionType.Identity

    # exp engine pattern: A=ACT, D=DVE, cycled per (super, pair) step
    exp_pat = os.environ.get("KERNEL_EXP_PAT", "AADAD")

    nc = bacc.Bacc("TRN2", target_bir_lowering=False, debug=False,
                   enable_asserts=False)

    # ---- DRAM ----
    def dt2(name, shape, dt):
        return nc.dram_tensor(name, shape, dt, kind="ExternalInput")[:, :]

    xfb_d = dt2("xfb", [128, 2 * HW], bf16)
    xfe_d = dt2("xfe", [128, 2 * HW], e4)
    xhb_d = dt2("xhb", [128, 2 * 80], bf16)
    wq_d = dt2("wq", [128, 2 * 2 * 128], e4)
    wk_d = dt2("wk", [128, 2 * 2 * 128], e4)
    wv8_d = dt2("wv8", [128, 2 * 256], e4)
    wvb_d = dt2("wvb", [128, 2 * 2 * 128], bf16)
    wdiag_d = dt2("wdiag", [128, 18 * 128], bf16)
    wdwb_d = dt2("wdwb", [10, CH], bf16)
    maug_d = dt2("maug", [10, NQ], bf16)
    wp_d = dt2("wp", [128, 2 * 2 * 128], bf16)
    eye_d = dt2("eye", [128, 128], bf16)
    bq_d = dt2("bq", [128, 2], mybir.dt.float32)
    bp_d = dt2("bp", [128, 2], mybir.dt.float32)
    out_d = nc.dram_tensor("out", [128, 2 * NQ], f32,
                           kind="ExternalOutput")[:, :]
    out_d3 = out_d.rearrange("p (t n) -> p t n", t=2)

    with tile.TileContext(nc) as tc:
        with tc.tile_pool(name="persist", bufs=1) as P:
            # ---- loads (sync queue: attention-critical first) ----
            xfe = P.tile([128, 2, HW], e4, name="xfe")
            wq = P.tile([128, 2, 2, 128], e4, name="wq")
            wk = P.tile([128, 2, 2, 128], e4, name="wk")
            wv8 = P.tile([128, 2, 256], e4, name="wv8")
            eye = P.tile([128, 128], bf16, name="eye")
            nc.sync.dma_start(out=eye, in_=eye_d)
            bq = P.tile([128, 2], f32, name="bq")
            bp = P.tile([128, 2], f32, name="bp")
            nc.sync.dma_start(out=xfe, in_=xfe_d.rearrange("p (j n) -> p j n", j=2))
            nc.sync.dma_start(out=wq, in_=wq_d.rearrange("p (j e m) -> p j e m", j=2, e=2))
            nc.sync.dma_start(out=wk, in_=wk_d.rearrange("p (j e m) -> p j e m", j=2, e=2))
            nc.sync.dma_start(out=wv8, in_=wv8_d.rearrange("p (j o) -> p j o", j=2))
            nc.sync.dma_start(out=bq, in_=bq_d)
            nc.sync.dma_start(out=bp, in_=bp_d)
            # vpad-conv inputs on scalar queue
            xfb = P.tile([128, 2, HW], bf16, name="xfb")
            xhb = P.tile([128, 2, 80], bf16, name="xhb")
            wvb = P.tile([128, 2, 2, 128], bf16, name="wvb")
            nc.scalar.dma_start(out=xfb, in_=xfb_d.rearrange("p (j n) -> p j n", j=2))
            nc.scalar.dma_start(out=xhb, in_=xhb_d.rearrange("p (j n) -> p j n", j=2))
            nc.scalar.dma_start(out=wvb, in_=wvb_d.rearrange("p (j t m) -> p j t m", j=2, t=2))
            # dw + proj weights on vector queue
            wdiag = P.tile([128, 18, 128], bf16, name="wdiag")
            wdwb = P.tile([10, CH], bf16, name="wdwb")
            maug = P.tile([10, NQ], bf16, name="maug")
            wp = P.tile([128, 2, 2, 128], bf16, name="wp")
            nc.sync.dma_start(out=wdiag, in_=wdiag_d.rearrange("p (t k) -> p t k", t=18))
            nc.sync.dma_start(out=wdwb, in_=wdwb_d)
            nc.sync.dma_start(out=maug, in_=maug_d)
            nc.sync.dma_start(out=wp, in_=wp_d.rearrange("p (j t o) -> p j t o", j=2, t=2))

            # ---- persistent working tensors ----
            K_dr = P.tile([128, 2, HW], e4, name="K_dr")
            Q_dr = P.tile([128, 2, NQ], e4, name="Q_dr")
            vt = P.tile([128, 7, 2, 4, 68], e4, name="vt")
            vpad = P.tile([128, 2, 22, 42], bf16, name="vpad")
            dw_sb = P.tile([128, 2, NQ], bf16, name="dw_sb")
            attn_out = P.tile([128, 2, NQ], bf16, name="attn_out")
            pre = P.tile([128, 2, NQ], bf16, name="pre")
            out_sb = P.tile([128, 2, NQ], f32, name="out_sb")
            ones64 = P.tile([1, 64], bf16, name="ones64")
            nc.vector.memset(ones64, 1.0)
            # warm the Exp table early (overlaps initial DMAs)
            actwarm = P.tile([1, 2], f32, name="actwarm")
            nc.scalar.activation(actwarm[0:1, 0:1], ones64[0:1, 0:1], Exp)
            # vpad zero borders (cols 0 and 41); rows 0/21 come from halo convs
            nc.gpsimd.memset(vpad[:, :, :, 0:1], 0.0)
            nc.gpsimd.memset(vpad[:, :, :, 41:42], 0.0)
            # ones column for the softmax denominator (slot 64 of every head)
            nc.gpsimd.memset(vt[:, :, :, :, 64:65], 1.0)

            # ================= F1: Q/K convs (fp8 DR) =================
            _fb = os.environ.get("KERNEL_FRONT_BUFS", "3,1,1").split(",")
            with tc.tile_pool(name="ps_qk", bufs=int(_fb[0]), space="PSUM") as psqk, \
                 tc.tile_pool(name="ps_vt", bufs=int(_fb[1]), space="PSUM") as psvt, \
                 tc.tile_pool(name="ps_dw", bufs=int(_fb[2]), space="PSUM") as psd:
                for lo, hi in Q_CHUNKS:
                    w = hi - lo
                    ps = psqk.tile([128, 2, 512], f32, tag="qk")
                    for e in range(2):
                        nc.tensor.matmul(ps[:, e, 0:w], wq[:, :, e, :],
                                         xfe[:, :, lo:hi], start=True, stop=True,
                                         perf_mode=DR, skip_group_check=True)
                    for e in range(2):
                        nc.scalar.activation(Q_dr[:, e, lo:hi], ps[:, e, 0:w],
                                             Ident, bias=bq[:, e:e + 1])
                for lo, hi in K_CHUNKS:
                    w = hi - lo
                    ps = psqk.tile([128, 2, 512], f32, tag="qk")
                    for e in range(2):
                        nc.tensor.matmul(ps[:, e, 0:w], wk[:, :, e, :],
                                         xfe[:, :, lo:hi], start=True, stop=True,
                                         perf_mode=DR, skip_group_check=True)
                    nc.scalar.copy(K_dr[:, :, lo:hi], ps[:, :, 0:w])

                # ================= F2: vT (fp8 DR) =================
                for s, k0, nk in SUPERS:
                    ps = psvt.tile([128, 2, 256], f32, tag="vt")
                    if nk == 256:
                        for j in range(2):
                            nc.tensor.matmul(ps[:, j, :],
                                             xfe[:, :, k0 + 128 * j:k0 + 128 * (j + 1)],
                                             wv8, start=(j == 0), stop=(j == 1),
                                             perf_mode=DR, skip_group_check=True)
                        nc.vector.tensor_copy(
                            vt[:, s, :, :, 0:64],
                            ps.rearrange("p j (h d) -> p j h d", h=4))
                    else:
                        nc.tensor.matmul(ps[0:64, 0, :], xfe[:, :, k0:k0 + nk],
                                         wv8, start=True, stop=True,
                                         perf_mode=DR, skip_group_check=True)
                        nc.vector.tensor_copy(
                            vt[0:64, s, 0, :, 0:64],
                            ps[0:64, 0, :].rearrange("p (h d) -> p h d", h=4))

                # ================= F3: vpad-V (bf16) + dw =================
                front_work = []

                def _vpad_unit(t, r0, r1, psd=psd):
                    lo, hi = 40 * r0, 40 * r1
                    ps = psd.tile([128, 512], f32, tag="dw")
                    for j in range(2):
                        nc.tensor.matmul(ps[:, 0:hi - lo], wvb[:, j, t, :],
                                         xfb[:, j, lo:hi],
                                         start=(j == 0), stop=(j == 1))
                    nc.vector.tensor_copy(
                        vpad[:, t, 1 + r0:1 + r1, 1:41],
                        ps[:, 0:hi - lo].rearrange("p (r c) -> p r c", c=40))

                def _halo_unit(t, psd=psd):
                    ps = psd.tile([128, 512], f32, tag="dw")
                    for j in range(2):
                        nc.tensor.matmul(ps[:, 0:80], wvb[:, j, t, :],
                                         xhb[:, j, :], start=(j == 0),
                                         stop=(j == 1))
                    nc.vector.tensor_copy(vpad[:, t, 0:1, 1:41], ps[:, 0:40])
                    nc.vector.tensor_copy(vpad[:, t, 21:22, 1:41], ps[:, 40:80])

                def _dw_unit(t, r0, r1, pool=None, tag="dw"):
                    nr = r1 - r0
                    if tag == "pa":
                        pst = pool.tile([128, 2, 2, 256], f32, tag=tag,
                                        name=f"dwps{t}_{r0}")
                        ps = pst[:, 0, :, :].rearrange("p j n -> p (j n)")
                    else:
                        ps = pool.tile([128, 512], f32, tag=tag,
                                       name=f"dwps{t}_{r0}")
                    first = True
                    for ky in range(3):
                        for kx in range(3):
                            nc.tensor.matmul(
                                ps[:, 0:nr * 40], wdiag[:, 9 * t + ky * 3 + kx, :],
                                vpad[:, t, r0 + ky:r1 + ky, kx:kx + 40],
                                start=first, stop=False)
                            first = False
                    nc.tensor.matmul(ps[:, 0:nr * 40],
                                     wdwb[:, 128 * t:128 * (t + 1)],
                                     maug[:, 40 * r0:40 * r1],
                                     start=False, stop=True)
                    nc.vector.tensor_copy(dw_sb[:, t, 40 * r0:40 * r1],
                                          ps[:, 0:nr * 40])

                for t in range(2):
                    for (r0, r1) in ((0, 11), (11, 20)):
                        front_work.append(lambda t=t, r0=r0, r1=r1: _vpad_unit(t, r0, r1))
                    front_work.append(lambda t=t: _halo_unit(t))
                defer_dw = os.environ.get("KERNEL_DEFER_DW", "0") == "1"
                for fw in front_work:
                    fw()
                front_work = []
                dw_units = [(t, r0, r1) for t in range(2)
                            for (r0, r1) in ((0, 12), (12, 20))]
                if not defer_dw:
                    for (t, r0, r1) in dw_units:
                        _dw_unit(t, r0, r1, pool=psd, tag="dw")
                    dw_units = []

            # ================= Attention =================
            assert not front_work
            pa_bufs = 3
            A_BIT = SCALE * 4.0 / LN2
            step_i = 0
            with tc.tile_pool(name="ps_pa", bufs=pa_bufs, space="PSUM") as pap, \
                 tc.tile_pool(name="ps_pavs", bufs=1, space="PSUM") as pvp, \
                 tc.tile_pool(name="ets", bufs=int(os.environ.get("KERNEL_ET_BUFS", "16"))) as etp, \
                 tc.tile_pool(name="rss", bufs=2) as rsp, \
                 tc.tile_pool(name="ats", bufs=2) as atp:
                norm_tail = [None]
                norm_stages = []  # deferred norm closures from previous chunk

                def _norm_stages(pavs, lo, hi):
                    w = hi - lo
                    nqt = (w + 127) // 128

                    def n1():
                        rs = rsp.tile([128, 2, 4, 1], f32, tag="rs", name="rs")
                        nc.vector.reciprocal(rs, pavs[:, :, :, 64:65])
                        at = atp.tile([128, 2, 256], bf16, tag="at", name="at")
                        na = int(os.environ.get("KERNEL_DRAIN_ACT", "2"))
                        for qt in range(nqt):
                            qw = min(128, w - 128 * qt)
                            for h in range(4):
                                if qt == 0 and h < na:
                                    nc.scalar.mul(
                                        at[0:qw, qt, 64 * h:64 * h + 64],
                                        pavs[0:qw, qt, h, 0:64],
                                        rs[0:qw, qt, h, :])
                                else:
                                    nc.vector.tensor_scalar_mul(
                                        at[0:qw, qt, 64 * h:64 * h + 64],
                                        pavs[0:qw, qt, h, 0:64],
                                        rs[0:qw, qt, h, :])

                        def n2():
                            # transpose back to [ch, q]; each transpose owns a
                            # full psum bank (zero-region = 2KB)
                            tps = []
                            for x in range(2):
                                tpf = pap.tile([128, 2, 2, 256], f32,
                                               tag="pa", name=f"tp{x}")
                                tps.append(tpf.bitcast(bf16))
                            for qt in range(nqt):
                                qw = min(128, w - 128 * qt)
                                for t in range(2):
                                    nc.tensor.transpose(
                                        tps[t][:, qt, 0, 0:qw],
                                        at[0:qw, qt, 128 * t:128 * (t + 1)],
                                        eye[0:qw, 0:qw])
                            for qt in range(nqt):
                                qw = min(128, w - 128 * qt)
                                l2 = lo + 128 * qt
                                for t in range(2):
                                    nc.vector.tensor_add(
                                        pre[:, t, l2:l2 + qw],
                                        tps[t][:, qt, 0, 0:qw],
                                        dw_sb[:, t, l2:l2 + qw])

                        def n3():
                            pp = pap.tile([128, 2, 2, 256], f32, tag="pa",
                                          name="pp")
                            for t in range(2):
                                for j in range(2):
                                    nc.tensor.matmul(pp[:, t, 0, 0:w],
                                                     wp[:, j, t, :],
                                                     pre[:, j, lo:hi],
                                                     start=(j == 0),
                                                     stop=(j == 1),
                                                     skip_group_check=True)
                            nob = int(os.environ.get("KERNEL_OB_DVE", "1"))
                            for t in range(2):
                                if t < nob:
                                    nc.vector.tensor_scalar_add(
                                        out_sb[:, t, lo:hi],
                                        pp[:, t, 0, 0:w], bp[:, t:t + 1])
                                else:
                                    nc.scalar.activation(out_sb[:, t, lo:hi],
                                                         pp[:, t, 0, 0:w],
                                                         Ident,
                                                         bias=bp[:, t:t + 1])
                            nc.sync.dma_start(out=out_d3[:, :, lo:hi],
                                              in_=out_sb[:, :, lo:hi])

                        norm_tail[0] = [n2, n3]

                    def nn2():
                        norm_tail[0][0]()

                    def nn3():
                        norm_tail[0][1]()

                    return [n1, nn2, nn3]

                for lo, hi in CHUNKS:
                    w = hi - lo
                    if w <= 36:
                        # packed small-chunk path: all supers in one pa/et per
                        # head-pair, one fat exp per pair
                        pavs = pvp.tile([128, 2, 4, 128], f32, tag="pavs")
                        pas, ets = [], []
                        for p2 in range(2):
                            if norm_stages:
                                norm_stages.pop(0)()
                            pa = pap.tile([128, 2, 2, 256], f32, tag="pa")
                            et = etp.tile([128, 2, 2, 256], e5, tag="et")
                            pas.append(pa)
                            ets.append(et)
                            for s, k0, nk in SUPERS:
                                for h2 in range(2):
                                    h = 2 * p2 + h2
                                    if nk == 256:
                                        for j in range(2):
                                            nc.tensor.matmul(
                                                pa[:, h2, j, 32 * s:32 * s + w],
                                                K_dr[32 * h:32 * h + 16, :,
                                                     k0 + 128 * j:k0 + 128 * (j + 1)],
                                                Q_dr[32 * h:32 * h + 16, :, lo:hi],
                                                start=(s == 0 and j == 0),
                                                stop=False,
                                                tile_position=(32 * h, 0),
                                                perf_mode=DR,
                                                skip_group_check=True)
                                    else:
                                        nc.tensor.matmul(
                                            pa[0:64, h2, 0, 32 * s:32 * s + w],
                                            K_dr[32 * h:32 * h + 16, :, k0:k0 + nk],
                                            Q_dr[32 * h:32 * h + 16, :, lo:hi],
                                            start=False, stop=True,
                                            tile_position=(32 * h, 0),
                                            perf_mode=DR,
                                            skip_group_check=True)
                            ns = 32 * len(SUPERS)
                            eng = exp_pat[step_i % len(exp_pat)]
                            step_i += 1
                            if eng == "A":
                                nc.scalar.activation(et[:, :, :, 0:ns],
                                                     pa[:, :, :, 0:ns], Exp,
                                                     scale=SCALE)
                            else:
                                nc.vector.tensor_scalar(
                                    et[:, :, :, 0:ns].bitcast(u8),
                                    pa[:, :, :, 0:ns], A_BIT, 60.0,
                                    op0=mybir.AluOpType.mult,
                                    op1=mybir.AluOpType.add)
                        while norm_stages:
                            norm_stages.pop(0)()
                        for p2 in range(2):
                            for s, k0, nk in SUPERS:
                                for h2 in range(2):
                                    h = 2 * p2 + h2
                                    start = (s == 0 and h == 0)
                                    stop = (s == 6 and h == 3)
                                    if nk == 256:
                                        nc.tensor.matmul(
                                            pavs[0:w, 0, h, 0:65],
                                            ets[p2][:, h2, :, 32 * s:32 * s + w],
                                            vt[:, s, :, h, 0:65],
                                            start=start, stop=stop,
                                            perf_mode=DR, skip_group_check=True)
                                    else:
                                        nc.tensor.matmul(
                                            pavs[0:w, 0, h, 0:65],
                                            ets[p2][0:64, h2, 0, 32 * s:32 * s + w],
                                            vt[0:64, s, 0, h, 0:65],
                                            start=start, stop=stop,
                                            skip_group_check=True)
                        norm_stages = _norm_stages(pavs, lo, hi)
                        continue
                    pavs = pvp.tile([128, 2, 4, 128], f32, tag="pavs")
                    pend = []
                    si = 0
                    for s, k0, nk in SUPERS:
                        for p2 in range(2):
                            # drain one deferred norm stage per step at the
                            # chunk head; AVs are held back until the old
                            # pavs has been fully read (muls done)
                            if dw_units:
                                t_, r0_, r1_ = dw_units.pop(0)
                                _dw_unit(t_, r0_, r1_, pool=pap, tag="pa")
                            elif norm_stages and si >= int(os.environ.get("KERNEL_NORM_SI", "1")):
                                norm_stages.pop(0)()
                            si += 1
                            pa = pap.tile([128, 2, 2, 256], f32, tag="pa")
                            et = etp.tile([128, 2, 2, 256], e5, tag="et")
                            if nk == 256:
                                for h2 in range(2):
                                    h = 2 * p2 + h2
                                    for j in range(2):
                                        nc.tensor.matmul(
                                            pa[:, h2, j, 0:w],
                                            K_dr[32 * h:32 * h + 16, :,
                                                 k0 + 128 * j:k0 + 128 * (j + 1)],
                                            Q_dr[32 * h:32 * h + 16, :, lo:hi],
                                            start=(j == 0), stop=(j == 1),
                                            tile_position=(32 * h, 0),
                                            perf_mode=DR, skip_group_check=True)
                                pa_in = pa[:, :, :, 0:w]
                                et_out = et[:, :, :, 0:w]
                            else:
                                for h2 in range(2):
                                    h = 2 * p2 + h2
                                    nc.tensor.matmul(
                                        pa[0:64, h2, 0, 0:w],
                                        K_dr[32 * h:32 * h + 16, :, k0:k0 + nk],
                                        Q_dr[32 * h:32 * h + 16, :, lo:hi],
                                        start=True, stop=True,
                                        tile_position=(32 * h, 0),
                                        perf_mode=DR, skip_group_check=True)
                                pa_in = pa[0:64, :, 0:1, 0:w]
                                et_out = et[0:64, :, 0:1, 0:w]
                            eng = exp_pat[step_i % len(exp_pat)]
                            step_i += 1
                            if eng == "A":
                                nc.scalar.activation(et_out, pa_in, Exp,
                                                     scale=SCALE)
                            else:
                                nc.vector.tensor_scalar(
                                    et_out.bitcast(u8), pa_in, A_BIT, 60.0,
                                    op0=mybir.AluOpType.mult,
                                    op1=mybir.AluOpType.add)
                            pend.append((s, nk, p2, et))
                            if not norm_stages and len(pend) > int(os.environ.get("KERNEL_AV_LAG", "3")):
                                _emit_av(nc, mybir, pavs, vt, pend.pop(0), w, lo)
                    while norm_stages:
                        norm_stages.pop(0)()
                    for pv in pend:
                        _emit_av(nc, mybir, pavs, vt, pv, w, lo)
                    norm_stages = _norm_stages(pavs, lo, hi)
                for st in norm_stages:
                    st()

    nc.finalize()
    return nc


def _emit_av(nc, mybir, pavs, vt, pend, w, lo):
    # flipped: out [queries, 65] so the denominator (col 64) is per-partition
    DR = mybir.MatmulPerfMode.DoubleRow
    s, nk, p2, et = pend
    nqt = (w + 127) // 128
    for h2 in range(2):
        h = 2 * p2 + h2
        for qt in range(nqt):
            qw = min(128, w - 128 * qt)
            start = (s == 0 and h == 0)
            stop = (s == 6 and h == 3)
            if nk == 256:
                nc.tensor.matmul(
                    pavs[0:qw, qt, h, 0:65],
                    et[:, h2, :, 128 * qt:128 * qt + qw],
                    vt[:, s, :, h, 0:65], start=start, stop=stop,
                    perf_mode=DR, skip_group_check=True)
            else:
                nc.tensor.matmul(
                    pavs[0:qw, qt, h, 0:65],
                    et[0:64, h2, 0, 128 * qt:128 * qt + qw],
                    vt[0:64, s, 0, h, 0:65], start=start, stop=stop,
                    skip_group_check=True)


def _prep_host(inputs):
    x = np.asarray(inputs["x"], np.float32)

    def fold(g, b, m, v):
        s = np.asarray(g, np.float32) / np.sqrt(np.asarray(v, np.float32) + EPS)
        return s, np.asarray(b, np.float32) - np.asarray(m, np.float32) * s

    s_qkv, b_qkv = fold(inputs["g_qkv"], inputs["b_qkv"], inputs["m_qkv"],
                        inputs["v_qkv"])
    Wt = np.asarray(inputs["w_qkv"], np.float32)[:, :, 0, 0] * s_qkv[:, None]

    # head h occupies partitions 32h..32h+16 (k' = d//2); e = parity of d
    sel = np.arange(128)
    hh, kp = sel // 32, sel % 32
    valid = kp < 16
    wq = np.zeros((128, 2, 2, 128), np.float32)
    wk = np.zeros((128, 2, 2, 128), np.float32)
    bqv = np.zeros((128, 2), np.float32)
    for e in range(2):
        cq = np.where(valid, hh * 128 + 2 * kp + e, 0)
        ck = np.where(valid, hh * 128 + 32 + 2 * kp + e, 0)
        for j in range(2):
            wq[:, j, e, :] = (Wt[cq, 128 * j:128 * (j + 1)] * valid[:, None]).T
            wk[:, j, e, :] = (Wt[ck, 128 * j:128 * (j + 1)] * valid[:, None]).T
        bqv[:, e] = b_qkv[cq] * valid
    o_idx = np.arange(256)
    colv = (o_idx // 64) * 128 + 64 + o_idx % 64
    Wv = Wt[colv]                                  # [256 vch, 256 in]
    wv8 = np.zeros((128, 2, 256), np.float32)
    wvb = np.zeros((128, 2, 2, 128), np.float32)
    for j in range(2):
        wv8[:, j, :] = Wv[:, 128 * j:128 * (j + 1)].T
        for t in range(2):
            wvb[:, j, t, :] = Wv[128 * t:128 * (t + 1), 128 * j:128 * (j + 1)].T
    bias_v = b_qkv[colv]

    s_dw, b_dw = fold(inputs["g_dw"], inputs["b_dw"], inputs["m_dw"],
                      inputs["v_dw"])
    wdw = np.asarray(inputs["w_dw"], np.float32)[:, 0].reshape(CH, 9) * \
        s_dw[:, None]
    wdiag = np.zeros((128, 2 * 9, 128), np.float32)
    for t in range(2):
        for tap in range(9):
            np.fill_diagonal(wdiag[:, 9 * t + tap, :],
                             wdw[128 * t:128 * (t + 1), tap])
    wdwb = np.zeros((10, CH), np.float32)
    wdwb[:9] = (wdw * bias_v[:, None]).T
    wdwb[9] = b_dw + bias_v

    s_pr, b_pr = fold(inputs["g_proj"], inputs["b_proj"], inputs["m_proj"],
                      inputs["v_proj"])
    Wp = np.asarray(inputs["w_proj"], np.float32)[:, :, 0, 0] * s_pr[:, None]
    wp = np.zeros((128, 2, 2, 128), np.float32)
    for j in range(2):
        for t in range(2):
            wp[:, j, t, :] = Wp[128 * t:128 * (t + 1), 128 * j:128 * (j + 1)].T
    bpv = np.stack([b_pr[0:128], b_pr[128:256]], axis=1)

    in_maps = []
    for core in range(NCORES):
        b, s = divmod(core, 2)
        xb = x[b].reshape(CH, HW)
        own = xb[:, s * NQ:(s + 1) * NQ]
        other = xb[:, (1 - s) * NQ:(2 - s) * NQ]
        xf2 = np.concatenate([own, other], axis=1)        # [256, 1600]
        xfp = np.stack([xf2[0:128], xf2[128:256]], axis=1)  # [128, 2, 1600]
        xh2 = np.zeros((CH, 80), np.float32)
        if s == 0:
            xh2[:, 40:80] = xb[:, 800:840]
        else:
            xh2[:, 0:40] = xb[:, 760:800]
        xhp = np.stack([xh2[0:128], xh2[128:256]], axis=1)
        maug = np.zeros((10, NQ), np.float32)
        gr = s * 20 + np.arange(20)[:, None] + np.zeros((1, 40), int)
        gc = np.zeros((20, 1), int) + np.arange(40)[None, :]
        for ky in range(3):
            for kx in range(3):
                inb = ((gr + ky - 1 >= 0) & (gr + ky - 1 <= 39) &
                       (gc + kx - 1 >= 0) & (gc + kx - 1 <= 39))
                maug[ky * 3 + kx] = inb.reshape(NQ).astype(np.float32)
        maug[9] = 1.0
        in_maps.append({
            "eye": np.eye(128, dtype=np.float32).astype(BF),
            "xfb": xfp.reshape(128, -1).astype(BF),
            "xfe": xfp.reshape(128, -1).astype(E4),
            "xhb": xhp.reshape(128, -1).astype(BF),
            "wq": wq.reshape(128, -1).astype(E4),
            "wk": wk.reshape(128, -1).astype(E4),
            "wv8": wv8.reshape(128, -1).astype(E4),
            "wvb": wvb.reshape(128, -1).astype(BF),
            "wdiag": wdiag.reshape(128, -1).astype(BF),
            "wdwb": wdwb.astype(BF),
            "maug": maug.astype(BF),
            "wp": wp.reshape(128, -1).astype(BF),
            "bq": bqv,
            "bp": bpv,
        })
    return in_maps


def kernel(**inputs):
    global _COMPILED, LAST_EXEC_NS, LAST_RESULTS
    from concourse import bass_utils

    if _COMPILED is None:
        _COMPILED = build_program()
    nc = _COMPILED
    in_maps = _prep_host(inputs)
    res = bass_utils.run_bass_kernel_spmd(
        nc, in_maps, core_ids=list(range(NCORES)), trace=False)
    LAST_EXEC_NS = res.exec_time_ns
    LAST_RESULTS = res
    y = np.zeros((B, CH, H, W), np.float32)
    for core in range(NCORES):
        b, s = divmod(core, 2)
        o = np.asarray(res.results[core]["out"], np.float32).reshape(128, 2, NQ)
        full = np.concatenate([o[:, 0, :], o[:, 1, :]], axis=0)  # [256, 800]
        y[b, :, s * 20:(s + 1) * 20, :] = full.reshape(CH, 20, 40)
    return y

